# revision 4
# baseline (speedup 1.0000x reference)
"""Binarized 3x3 conv (stride 1, pad 1) + bias on 8 Trainium2 NeuronCores.

Full problem: x[32,256,56,56] f32, weight[256,256,3,3] f32, bias[256] f32
-> y[32,256,56,56] f32 with y = conv2d(sign(x), sign(weight), pad=1) + bias
(sign(t) = +1 for t >= 0 else -1).

Sharding: data-parallel over batch. Each of the 8 cores gets 4 images and a
replicated copy of weight/bias, computes its shard fully on-device, and the
host concatenates the 8 output shards.

Per-core kernel (v2 — weight-stationary conv stream):
  - binarize x and w to +/-0.5 with one fused DVE op each ((v>=0) - 0.5);
    the final PSUM->SBUF copy applies scale=4 to undo the 0.25 product
    scale, so results are exactly the +/-1 conv (all integers, exact in f32).
  - x lives zero-padded in SBUF as [128(ci_p), 2(ci_blk), 3376] fp8 per
    image: 58x58 padded image rows + 1 guard element front/back.
  - weight is binarized to bf16, transposed on the PE (36 x 128x128
    transposes via identity, 3 taps per PSUM tile), and stored as fp8 lhsT
    [128(ci_p), 2(ci_blk), 9(tap), 256(co)].
  - conv: tap-outer, chunk-inner. For each (image, co_blk) the 7 row-chunks
    are split into groups {0..3} and {4..6}; within a group the loop runs
    taps outer, chunks inner, accumulating the 9 taps of each chunk in its
    own PSUM bank. The first matmul of each (tap, group) self-loads the
    stationary weights; the rest are marked ldweights=False so the PE skips
    the per-matmul weight reload (61 ns each on HW).
  - PSUM -> SBUF via ScalarE: Identity(psum*4 + bias[co]) into a per-
    (image,co_blk) [128, 3136] staging tile; one y DMA per group (7-KB
    descriptors instead of 1.8-KB ones).
"""

import numpy as np

import concourse.bacc as bacc
import concourse.mybir as mybir
import concourse.tile as tile
from concourse.bass_utils import run_bass_kernel_spmd
from concourse.masks import make_identity

F32 = mybir.dt.float32
BF16 = mybir.dt.bfloat16
FP8 = mybir.dt.float8e4
AF = mybir.ActivationFunctionType
ALU = mybir.AluOpType
DR = mybir.MatmulPerfMode.DoubleRow

N_CORES = 8
H = W = 56
WP = 58            # padded row width
CIN = 256
COUT = 256
CI_BLKS = 2        # 256 ci = 2 x 128 partitions
CO_BLKS = 2
R = 8              # output rows per chunk
NCHUNK = H // R    # 7
NV = R * WP        # 464 matmul moving free size
IMG_FA = 3376      # aligned per-ci_blk padded image elems (58*58+2 -> 3376)
GROUPS = ((0, 1, 2, 3), (4, 5, 6))


def _build_conv(tc, y_ap, x_ap, w_ap, b_ap, n_imgs):
    nc = tc.nc
    scale = 4.0  # undo (+/-0.5)*(+/-0.5) = +/-0.25 product scale

    with (
        tc.tile_pool(name="consts", bufs=1) as consts,
        tc.tile_pool(name="wstage", bufs=1) as wstage_pool,
        tc.tile_pool(name="lhst", bufs=1) as lhst_pool,
        tc.tile_pool(name="xstage", bufs=2) as xstage_pool,
        tc.tile_pool(name="xpad", bufs=1) as xpad_pool,
        tc.tile_pool(name="outsb", bufs=2) as out_pool,
        tc.tile_pool(name="psum", bufs=8, space="PSUM") as psum_pool,
    ):
        # --- constants -----------------------------------------------------
        ident = consts.tile([128, 128], BF16)
        make_identity(nc, ident)
        junk = consts.tile([128, 512], BF16, name="junk")
        nc.gpsimd.memset(junk, 0.0)

        wstage = wstage_pool.tile([128, CO_BLKS, CIN, 9], F32)
        wb = wstage_pool.tile([128, CO_BLKS, CIN, 9], BF16)
        lhst = lhst_pool.tile([128, CI_BLKS, 9, COUT], FP8)
        xstage0 = xstage_pool.tile([128, CI_BLKS, H * W], F32,
                                   name="xstage0", tag="xstage")

        def dma_w(c, b):
            # one quarter of the weights: co block c, ci block b
            nc.sync.dma_start(
                out=wstage[:, c, b * 128:(b + 1) * 128],
                in_=w_ap[c * 128:(c + 1) * 128, b * 128:(b + 1) * 128].rearrange(
                    "co ci kh kw -> co ci (kh kw)"),
            )

        def dma_x(xstage, n, r0, r1, b):
            nc.sync.dma_start(
                out=xstage[:, b, r0 * W:r1 * W],
                in_=x_ap[n, b * 128:(b + 1) * 128, r0:r1]
                    .rearrange("c h w -> c (h w)"),
            )

        # DMA issue order is bandwidth-critical: conv group A of (img 0, c=0)
        # can start once W_c0 + x rows 0..32 are in SBUF.
        dma_w(0, 0)
        dma_w(0, 1)
        dma_x(xstage0, 0, 0, 33, 0)
        dma_x(xstage0, 0, 0, 33, 1)
        dma_w(1, 0)
        dma_w(1, 1)
        dma_x(xstage0, 0, 33, H, 0)
        dma_x(xstage0, 0, 33, H, 1)
        bias_sb = consts.tile([128, CO_BLKS], F32)
        nc.scalar.dma_start(out=bias_sb, in_=b_ap.rearrange("(b p) -> p b", p=128))

        # --- weight prep ---------------------------------------------------
        def binz(dst, src):
            nc.vector.tensor_scalar(dst, src, 0.0, 0.5, ALU.is_ge, ALU.subtract)

        def junk_mm():
            # throwaway matmul on zeros; keeps the HAM clock gate from
            # throttling the PE while it waits for weights/input DMA
            jps = psum_pool.tile([128, 512], F32, name="ps", tag="ps")
            nc.tensor.matmul(jps, junk[:, :128], junk, start=True, stop=True)

        def wbinz(c):
            for b in range(CI_BLKS):
                binz(wb[:, c, b * 128:(b + 1) * 128],
                     wstage[:, c, b * 128:(b + 1) * 128])

        def wprep(c):
            # transpose 18 taps of co block c on the PE, 2 taps per PSUM
            # tile, one ScalarE PSUM->SBUF cast copy per pair
            for b in range(CI_BLKS):
                for t0 in range(0, 9, 3):
                    nt = min(3, 9 - t0)
                    tp = psum_pool.tile([128, 3, 128], BF16, name="ps", tag="ps")
                    for i in range(nt):
                        nc.tensor.transpose(
                            tp[:, i], wb[:, c, b * 128:(b + 1) * 128, t0 + i],
                            ident)
                    nc.scalar.copy(
                        out=lhst[:, b, t0:t0 + nt, c * 128:(c + 1) * 128],
                        in_=tp[:, 0:nt])

        # --- x buffers: persistent padded buffers, pad zeros written once
        NXPAD = 3
        xpads = [xpad_pool.tile([128, CI_BLKS, IMG_FA], FP8,
                                name=f"xpad{i}", tag=f"xpad{i}")
                 for i in range(NXPAD)]
        for xp in xpads:
            for b in range(CI_BLKS):
                # head guard + top pad row (+ first in-row pad col): elems 0..59
                nc.vector.memset(xp[:, b, 0:60], 0.0)
                # bottom pad row + tail guard: elems 1+57*58 .. 3375
                nc.vector.memset(xp[:, b, 1 + 57 * WP:IMG_FA], 0.0)
                # per-row right+left pad pairs at (1+h*58+57, 1+h*58+58)
                nc.vector.memset(
                    xp[:, b, 58:58 + 57 * WP].rearrange(
                        "p (h w) -> p h w", w=WP)[:, :, 0:2],
                    0.0,
                )

        # --- per-image pipeline -------------------------------------------
        def binz_x(xstage, xpad, r0, r1, b):
            # data rows: padded row h+1, cols 1..56
            dst = xpad[:, b, 60:60 + H * WP].rearrange(
                "p (h w) -> p h w", w=WP)[:, r0:r1, 0:W]
            src = xstage[:, b].rearrange("p (h w) -> p h w", w=W)[:, r0:r1]
            binz(dst, src)

        def conv_group(n, xpad, c, ks, osb):
            pss = [psum_pool.tile([128, NV], F32, name="ps", tag="ps")
                   for _ in ks]
            for t in range(9):
                kh, kw = divmod(t, 3)
                lw = lhst[:, 0:2, t, c * 128:(c + 1) * 128]
                for i, k in enumerate(ks):
                    base = (R * k + kh) * WP + kw  # incl. -1 guard shift
                    mm = nc.tensor.matmul(
                        pss[i],
                        lw,
                        xpad[:, 0:2, base:base + NV],
                        start=(t == 0),
                        stop=(t == 8),
                        perf_mode=DR,
                    )
                    if i > 0:
                        # reuse the stationary loaded by the i == 0 matmul
                        mm.ins.ldweights = False
            for i, k in enumerate(ks):
                nc.scalar.activation(
                    out=osb[:, R * W * k:R * W * (k + 1)].rearrange(
                        "p (r w) -> p r w", w=W),
                    in_=pss[i].rearrange("p (r w) -> p r w", w=WP)[:, :, 1:57],
                    func=AF.Identity,
                    bias=bias_sb[:, c:c + 1],
                    scale=scale,
                )
            lo, hi = R * W * ks[0], R * W * (ks[-1] + 1)
            nc.sync.dma_start(
                out=y_ap[n, c * 128:(c + 1) * 128]
                    .rearrange("co h w -> co (h w)")[:, lo:hi],
                in_=osb[:, lo:hi],
            )

        def load_image(n):
            # loads + binarizes image n into its xpad buffer
            xstage = xstage_pool.tile([128, CI_BLKS, H * W], F32,
                                      name=f"xstage{n}", tag="xstage")
            xpad = xpads[n % NXPAD]
            for r0, r1 in ((0, 28), (28, H)):
                for b in range(CI_BLKS):
                    dma_x(xstage, n, r0, r1, b)
                    binz_x(xstage, xpad, r0, r1, b)

        for n in range(n_imgs):
            xpad = xpads[n % NXPAD]
            if n == 0:
                # DVE order = dependency-critical order: wb c0 gates the c0
                # transposes, x rows 0..32 gate conv group A
                wbinz(0)
                binz_x(xstage0, xpad, 0, 33, 0)
                binz_x(xstage0, xpad, 0, 33, 1)
                wbinz(1)
                binz_x(xstage0, xpad, 33, H, 0)
                binz_x(xstage0, xpad, 33, H, 1)
                # PE order: junk warm-up while the w DMA lands, then the
                # transposes, then more junk until conv data is ready
                for _ in range(14):
                    junk_mm()
                wprep(0)
                for _ in range(6):
                    junk_mm()
                wprep(1)
            # prefetch image n+1 before image n's conv groups so its input
            # DMAs take queue priority over image n's output-DMA burst
            if n + 1 < n_imgs:
                load_image(n + 1)
            for c in range(CO_BLKS):
                osb = out_pool.tile([128, H * W], F32, name="osb")
                for ks in GROUPS:
                    conv_group(n, xpad, c, ks, osb)


_NC_CACHE = {}


def _get_nc(n_imgs):
    if n_imgs not in _NC_CACHE:
        nc = bacc.Bacc("TRN2", target_bir_lowering=False, debug=False)
        x_ap = nc.dram_tensor("x", [n_imgs, CIN, H, W], F32,
                              kind="ExternalInput").ap()
        w_ap = nc.dram_tensor("weight", [COUT, CIN, 3, 3], F32,
                              kind="ExternalInput").ap()
        b_ap = nc.dram_tensor("bias", [COUT], F32, kind="ExternalInput").ap()
        y_ap = nc.dram_tensor("y", [n_imgs, COUT, H, W], F32,
                              kind="ExternalOutput").ap()
        with tile.TileContext(nc) as tc:
            _build_conv(tc, y_ap, x_ap, w_ap, b_ap, n_imgs)
        nc.compile()
        _NC_CACHE[n_imgs] = nc
    return _NC_CACHE[n_imgs]


def kernel(x: np.ndarray, weight: np.ndarray, bias: np.ndarray) -> np.ndarray:
    assert x.shape[1:] == (CIN, H, W), x.shape
    assert x.shape[0] % N_CORES == 0, x.shape
    n_imgs = x.shape[0] // N_CORES
    x = np.ascontiguousarray(x, dtype=np.float32)
    weight = np.ascontiguousarray(weight, dtype=np.float32)
    bias = np.ascontiguousarray(bias, dtype=np.float32)

    nc = _get_nc(n_imgs)
    shards = [x[i * n_imgs:(i + 1) * n_imgs] for i in range(N_CORES)]
    in_maps = [{"x": s, "weight": weight, "bias": bias} for s in shards]
    res = run_bass_kernel_spmd(nc, in_maps, core_ids=list(range(N_CORES)))
    return np.concatenate([r["y"] for r in res.results], axis=0)
